# revision 50
# baseline (speedup 1.0000x reference)
"""GAT message-passing kernel for Trainium2, 8 NeuronCores, dst-aligned.

Strategy (self-contained; sized for N=50000, E=800000, D=128, H=4, C=16,
ED=64 but parameterized):
 - Nodes are sorted by in-degree and packed 128-consecutive into windows,
   so a window's max degree ~= its mean degree.  Window w's edges live in
   a [128 partitions x C_w columns] slot grid: partition = destination
   node's slot, column = edge ordinal.  Windows are dealt round-robin to
   the 8 cores with an equalized column schedule CS (max over the 8 cores
   at each rank) so every core runs the identical SPMD program.
 - The host ships, per edge slot, the source node's raw features x[src]
   (bf16, transposed) and edge_attr plus a mask row (-256 for padded
   slots).  No device-side gather, no index tensors: the layout IS the
   graph.
 - Per 128-edge block two bf16 matmuls write one PSUM tile: [xh | a_src]
   = x_src @ [W.T|u_src.T|0] and, accumulated onto cols 64:72,
   [v.ea+mask | v.ea] (mask row kills padded slots).  a_dst is a
   per-partition broadcast from the core's own node table.  alpha =
   lrelu(a_src+a_dst+a_edge) -> exp (ScalarE); softmax denominator and
   sum-of-a_edge via single 3D tensor_reduce ops; messages = one
   broadcast multiply + a halving-tree reduction, all packed bf16 SBUF.
 - The emission order is software-pipelined (stage B(w) on PE/ACT, stage
   C(w-1) alpha/exp, stage D(w-2) messages/close) so no in-order engine
   queue head-blocks on a cross-engine dependency.
 - Self-loops (PyG GATConv default: loop edge_attr = per-dst mean of
   incoming edge_attr) close each window via the pure sum-of-a_edge.
"""

import math

import numpy as np

NCORES = 8
D_IN = 128
H_HEADS = 4
C_OUT = 16
HC = H_HEADS * C_OUT  # 64
ED_DIM = 64
EAR = ED_DIM + 1      # edge-attr rows + mask row
NEG_SLOPE = 0.2
MASKV = -256.0        # padded slots: exp(lrelu(-256+...)) == 0
XU = 72               # psum row: [xh(64) | asrc+aedge+mask(4) | aedge(4)]
W80 = 80              # Wx cols: [W | u_src | 0 | u_dst | u_src+u_dst]

P = 128

TRACE = False       # set by test harness to capture an NTFF profile
LAST_RESULT = None  # BassKernelResults of the last traced run


class _Cfg:
    def __init__(self, nwl, cs):
        self.NWL = nwl            # windows per core
        self.CS = tuple(cs)       # 128-edge blocks per window (shared SPMD)
        self.TOTB = sum(cs)       # total blocks per core
        self.CMAX = max(cs)
        self.PCR = nwl * P        # node slots per core

    def key(self):
        return (self.NWL, self.CS)


def _fold_weights(W, W_edge, att_src, att_dst, att_edge):
    H, C = att_src.shape
    D = W.shape[1]
    ED = W_edge.shape[1]
    u_src = np.einsum("hc,hcd->hd", att_src, W.reshape(H, C, D))
    u_dst = np.einsum("hc,hcd->hd", att_dst, W.reshape(H, C, D))
    v = np.einsum("hc,hcd->hd", att_edge, W_edge.reshape(H, C, ED))
    Wx = np.zeros((D, W80), np.float32)
    Wx[:, :HC] = W.T
    Wx[:, HC:HC + H] = u_src.T
    Wx[:, HC + 2 * H:HC + 3 * H] = u_dst.T
    Wx[:, HC + 3 * H:] = (u_src + u_dst).T
    # vTm: cols 0:4 = v.T with mask row MASKV; cols 4:8 = v.T with mask 0
    vTm = np.zeros((EAR, 2 * H), np.float32)
    vTm[:ED, :H] = v.T
    vTm[:ED, H:] = v.T
    vTm[ED, :H] = MASKV
    return Wx, vTm


def _prep(x, src, dst, edge_attr):
    """Degree-sorted dst-aligned slot layout; per-core input slabs."""
    from concourse import mybir

    bf16 = mybir.dt.np(mybir.dt.bfloat16)
    n = x.shape[0]
    nwl = math.ceil(n / (P * NCORES))
    nwin = NCORES * nwl

    deg = np.bincount(dst, minlength=n).astype(np.int64)
    order = np.argsort(-deg, kind="stable")
    node_win = np.empty(n, np.int32)
    node_slot = np.empty(n, np.int32)
    ranks = np.arange(n, dtype=np.int64)
    node_win[order] = (ranks // P).astype(np.int32)
    node_slot[order] = (ranks % P).astype(np.int32)
    # window w max degree = degree of its first (highest-degree) node
    wmax = deg[order[np.minimum(np.arange(nwin) * P, n - 1)]]
    cs = np.maximum(wmax[0::NCORES], 1).astype(np.int64)  # equalized (desc)
    # processing order: a small window first (fast pipeline warmup) and a
    # small one last (fast drain); the big ones run in the saturated middle
    if nwl >= 3:
        order_groups = [nwl - 2] + list(range(nwl - 2)) + [nwl - 1]
    else:
        order_groups = list(range(nwl))
    cs_sched = [int(cs[g]) for g in order_groups]
    inv_groups = np.empty(nwl, np.int64)
    inv_groups[np.array(order_groups)] = np.arange(nwl)
    cfg = _Cfg(nwl, cs_sched)
    cb = np.zeros(nwl + 1, np.int64)
    np.cumsum(np.array(cs_sched, np.int64), out=cb[1:])
    totb = cfg.TOTB

    core_of_win = np.arange(nwin) % NCORES
    lw_of_win = inv_groups[np.arange(nwin) // NCORES]

    # edge -> (core, column in core slab, partition)
    ew = node_win[dst]
    ep = node_slot[dst]
    eorder = np.argsort(dst, kind="stable")
    ds = dst[eorder]
    first = np.zeros(len(ds), bool)
    first[0] = True
    first[1:] = ds[1:] != ds[:-1]
    gidx = np.flatnonzero(first)
    ec = np.arange(len(ds), dtype=np.int64)
    ec -= np.repeat(ec[gidx], np.diff(np.append(gidx, len(ds))))
    ecol = np.empty(len(ds), np.int64)
    ecol[eorder] = ec                                  # ordinal within dst
    ecore = core_of_win[ew]
    eslab = (cb[lw_of_win[ew]] + ecol) * P + ep        # column in core slab

    x_bf = np.ascontiguousarray(x.astype(bf16))
    ea_bf = np.ascontiguousarray(edge_attr.astype(bf16))

    in_maps = []
    for c in range(NCORES):
        em = ecore == c
        cols = eslab[em]
        xe = np.zeros((totb * P, D_IN), bf16)
        xe[cols] = x_bf[src[em]]
        xeT = np.ascontiguousarray(xe.T)
        eat = np.zeros((EAR, totb * P), bf16)
        eat[ED_DIM, :] = 1.0
        ea_blk = np.zeros((totb * P, ED_DIM), bf16)
        ea_blk[cols] = ea_bf[em]
        eat[:ED_DIM] = ea_blk.T
        eat[ED_DIM, cols] = 0.0

        wins = np.array([g * NCORES + c for g in order_groups])  # lw order
        xs = np.zeros((cfg.PCR, D_IN), bf16)
        invc = np.ones((P, nwl), np.float32)
        for lw, w in enumerate(wins):
            base = w * P
            cnt = min(P, n - base) if base < n else 0
            nd = order[base:base + cnt]
            xs[lw * P:lw * P + cnt] = x_bf[nd]
            invc[:cnt, lw] = 1.0 / np.maximum(deg[nd], 1.0)
        xsT = np.ascontiguousarray(xs.T)
        in_maps.append(dict(xeT=xeT, eaT=eat, xsT=xsT,
                            invc=np.ascontiguousarray(invc)))

    winpos = (core_of_win[node_win].astype(np.int64) * cfg.PCR
              + lw_of_win[node_win].astype(np.int64) * P + node_slot)
    meta = dict(winpos=winpos)
    return cfg, in_maps, meta


def _build_nc(cfg):
    import concourse.tile as tile
    from concourse import bacc, mybir
    from contextlib import ExitStack

    f32 = mybir.dt.float32
    bf16 = mybir.dt.bfloat16
    NWL, CS, TOTB, CMAX, PCR = cfg.NWL, cfg.CS, cfg.TOTB, cfg.CMAX, cfg.PCR
    UH = H_HEADS
    G = 7  # blocks per PSUM group: 7*72*4B = 2016B, one bank

    nc = bacc.Bacc("TRN2", target_bir_lowering=False, debug=False,
                   num_devices=NCORES)
    xeT = nc.dram_tensor("xeT", [P, TOTB * P], bf16, kind="ExternalInput").ap()
    eaT = nc.dram_tensor("eaT", [EAR, TOTB * P], bf16,
                         kind="ExternalInput").ap()
    xsT = nc.dram_tensor("xsT", [P, PCR], bf16, kind="ExternalInput").ap()
    Wx = nc.dram_tensor("Wx", [P, W80], bf16, kind="ExternalInput").ap()
    vTm = nc.dram_tensor("vTm", [EAR, 2 * UH], bf16,
                         kind="ExternalInput").ap()
    invc = nc.dram_tensor("invc", [P, NWL], f32, kind="ExternalInput").ap()
    out = nc.dram_tensor("out", [PCR, HC], f32, kind="ExternalOutput").ap()

    AF = mybir.ActivationFunctionType
    ALU = mybir.AluOpType
    AX = mybir.AxisListType

    with tile.TileContext(nc) as tc, ExitStack() as ctx:
        cpool = ctx.enter_context(tc.tile_pool(name="const", bufs=1))
        selfpool = ctx.enter_context(tc.tile_pool(name="selfr", bufs=1))
        xspool = ctx.enter_context(tc.tile_pool(name="xs", bufs=1))
        xepool = ctx.enter_context(tc.tile_pool(name="xe", bufs=3))
        eapool = ctx.enter_context(tc.tile_pool(name="ea", bufs=3))
        xhpool = ctx.enter_context(tc.tile_pool(name="xh", bufs=3))
        alfpool = ctx.enter_context(tc.tile_pool(name="alf", bufs=3))
        wpool = ctx.enter_context(tc.tile_pool(name="win", bufs=3))
        mpool = ctx.enter_context(tc.tile_pool(name="m", bufs=2))
        opool = ctx.enter_context(tc.tile_pool(name="o", bufs=3))
        psself = ctx.enter_context(
            tc.tile_pool(name="ps_s", bufs=2, space="PSUM"))
        psx = ctx.enter_context(tc.tile_pool(name="ps_x", bufs=4,
                                             space="PSUM"))
        psacc = ctx.enter_context(tc.tile_pool(name="ps_acc", bufs=2,
                                               space="PSUM"))

        Wx_sb = cpool.tile([P, W80], bf16)
        nc.sync.dma_start(Wx_sb[:], Wx[:])
        from concourse.masks import make_identity
        ident_sb = cpool.tile([P, P], bf16)
        make_identity(nc, ident_sb[:])
        vTm_sb = cpool.tile([EAR, 2 * UH], bf16)
        nc.sync.dma_start(vTm_sb[:], vTm[:])
        invc_sb = cpool.tile([P, NWL], f32)
        nc.sync.dma_start(invc_sb[:], invc[:])
        selfr = selfpool.tile([P, NWL * W80], f32)
        xs = xspool.tile([P, PCR], bf16)

        # ---- software-pipelined main loop ----
        cbs = [0]
        for c in CS:
            cbs.append(cbs[-1] + c)
        state = {}

        def stage_a(w):  # input slab DMA
            C = CS[w]
            cb = cbs[w]
            xe = xepool.tile([P, CMAX * P], bf16, tag="xe")
            nc.sync.dma_start(xe[:, :C * P], xeT[:, cb * P:(cb + C) * P])
            ea = eapool.tile([EAR, CMAX * P], bf16, tag="ea")
            nc.gpsimd.dma_start(ea[:, :C * P], eaT[:, cb * P:(cb + C) * P])
            state[w] = dict(xe=xe, ea=ea)

        def stage_b(w):  # matmuls + PSUM->SBUF copies
            C = CS[w]
            st = state[w]
            xe, ea = st["xe"], st["ea"]
            xhs = xhpool.tile([P, CMAX * HC], bf16, tag="xhs")
            alf = alfpool.tile([P, CMAX * 2 * UH], f32, tag="alf")
            for g in range(math.ceil(C / G)):
                c0 = g * G
                ng = min(G, C - c0)
                ps = psx.tile([P, G * XU], f32)
                for i in range(ng):
                    c = c0 + i
                    nc.tensor.matmul(
                        out=ps[:, i * XU:i * XU + XU],
                        lhsT=xe[:, c * P:(c + 1) * P],
                        rhs=Wx_sb[:, 0:XU], start=True, stop=False,
                        skip_group_check=True)
                    nc.tensor.matmul(
                        out=ps[:, i * XU + HC:i * XU + XU],
                        lhsT=ea[:, c * P:(c + 1) * P],
                        rhs=vTm_sb[:], start=False, stop=True,
                        skip_group_check=True)
                psv = ps[:, :ng * XU].rearrange("p (c u) -> p c u", u=XU)
                nc.scalar.activation(
                    xhs[:, c0 * HC:(c0 + ng) * HC], psv[:, :, 0:HC],
                    AF.Copy)
                nc.scalar.activation(
                    alf[:, c0 * 2 * UH:(c0 + ng) * 2 * UH],
                    psv[:, :, HC:XU], AF.Copy)
            pss = psself.tile([P, W80], f32)
            nc.tensor.matmul(out=pss[:], lhsT=xs[:, w * P:(w + 1) * P],
                             rhs=Wx_sb[:], start=True, stop=True)
            nc.scalar.activation(selfr[:, w * W80:(w + 1) * W80], pss[:],
                                 AF.Copy)
            st["xhs"] = xhs
            st["alf"] = alf

        def stage_c(w):  # alpha: +adst, lrelu, exp, den, aes
            C = CS[w]
            st = state[w]
            alf = st["alf"]
            selfw = selfr[:, w * W80:(w + 1) * W80]
            al8 = alf[:, :2 * UH * C].rearrange("p (c h) -> p c h",
                                                h=2 * UH)
            nc.gpsimd.tensor_tensor(
                out=al8[:, :, 0:UH], in0=al8[:, :, 0:UH],
                in1=selfw[:, HC + 2 * UH:HC + 3 * UH].unsqueeze(1)
                .broadcast_to([P, C, UH]), op=ALU.add)
            lrt = wpool.tile([P, CMAX * UH], f32, tag="lrt")
            lrtv = lrt[:, :UH * C].rearrange("p (c h) -> p c h", h=UH)
            nc.vector.scalar_tensor_tensor(
                out=lrtv, in0=al8[:, :, 0:UH], scalar=NEG_SLOPE,
                in1=al8[:, :, 0:UH], op0=ALU.mult, op1=ALU.max)
            expal = wpool.tile([P, CMAX * UH], bf16, tag="expal")
            nc.scalar.activation(expal[:, :UH * C], lrt[:, :UH * C], AF.Exp)
            den = wpool.tile([P, UH], f32, tag="den")
            nc.vector.tensor_reduce(
                den[:], expal[:, :UH * C].rearrange("p (c h) -> p h c",
                                                    h=UH),
                axis=AX.X, op=ALU.add)
            aes = wpool.tile([P, UH], f32, tag="aes")
            nc.vector.tensor_reduce(
                aes[:], al8[:, :, UH:2 * UH].rearrange("p c h -> p h c"),
                axis=AX.X, op=ALU.add)
            st["expal"] = expal
            st["den"] = den
            st["aes"] = aes

        def stage_d(w):  # messages + close
            C = CS[w]
            st = state.pop(w)
            xhs, expal = st["xhs"], st["expal"]
            den, aes = st["den"], st["aes"]
            selfw = selfr[:, w * W80:(w + 1) * W80]
            mw = mpool.tile([P, CMAX * HC], bf16, tag="mw")
            nc.vector.tensor_tensor(
                out=mw[:, :C * HC].rearrange("p (c h u) -> p c h u",
                                             h=UH, u=C_OUT),
                in0=xhs[:, :C * HC].rearrange("p (c h u) -> p c h u",
                                              h=UH, u=C_OUT),
                in1=expal[:, :UH * C].rearrange("p (c h) -> p c h", h=UH)
                .unsqueeze(3).broadcast_to([P, C, UH, C_OUT]),
                op=ALU.mult)
            JA = min(4, C)
            nga = math.ceil(C / 4)
            psa = psacc.tile([P, 4 * HC], f32)
            for g in range(nga):
                c0 = 4 * g
                nq = min(4, C - c0)
                nc.tensor.matmul(
                    out=psa[:, :nq * HC], lhsT=ident_sb[:],
                    rhs=mw[:, c0 * HC:(c0 + nq) * HC],
                    start=(g == 0), stop=(g == nga - 1),
                    skip_group_check=True)
            acc = opool.tile([P, HC], f32, tag="acc")
            if JA == 1:
                nc.vector.tensor_copy(acc[:], psa[:, :HC])
            else:
                nc.vector.tensor_reduce(
                    acc[:], psa[:, :JA * HC].rearrange("p (j u) -> p u j",
                                                       u=HC),
                    axis=AX.X, op=ALU.add)

            lae = wpool.tile([P, UH], f32, tag="lae")
            nc.vector.tensor_scalar(
                out=lae[:], in0=aes[:], scalar1=invc_sb[:, w:w + 1],
                scalar2=None, op0=ALU.mult)
            asf = wpool.tile([P, UH], f32, tag="asf")
            nc.gpsimd.tensor_tensor(out=asf[:],
                                    in0=selfw[:, HC + 3 * UH:W80],
                                    in1=lae[:], op=ALU.add)
            es = wpool.tile([P, UH], f32, tag="es")
            nc.vector.scalar_tensor_tensor(
                out=es[:], in0=asf[:], scalar=NEG_SLOPE, in1=asf[:],
                op0=ALU.mult, op1=ALU.max)
            nc.scalar.activation(es[:], es[:], AF.Exp)
            dent = wpool.tile([P, UH], f32, tag="dent")
            nc.gpsimd.tensor_tensor(out=dent[:], in0=es[:], in1=den[:],
                                    op=ALU.add)
            rec = wpool.tile([P, UH], f32, tag="rec")
            nc.vector.reciprocal(rec[:], dent[:])
            ot = opool.tile([P, HC], f32, tag="ot")
            nc.vector.tensor_tensor(
                out=ot[:].rearrange("p (h u) -> p h u", u=C_OUT),
                in0=selfw[:, 0:HC].rearrange("p (h u) -> p h u", u=C_OUT),
                in1=es[:].unsqueeze(2).broadcast_to([P, UH, C_OUT]),
                op=ALU.mult)
            nc.gpsimd.tensor_tensor(out=ot[:], in0=ot[:], in1=acc[:],
                                    op=ALU.add)
            nc.gpsimd.tensor_tensor(
                out=ot[:].rearrange("p (h u) -> p h u", u=C_OUT),
                in0=ot[:].rearrange("p (h u) -> p h u", u=C_OUT),
                in1=rec[:].unsqueeze(2).broadcast_to([P, UH, C_OUT]),
                op=ALU.mult)
            nc.sync.dma_start(out[w * P:(w + 1) * P, :], ot[:])

        stage_a(0)
        if NWL > 1:
            stage_a(1)
        nc.sync.dma_start(xs[:], xsT[:])
        for w in range(NWL):
            if w + 2 < NWL:
                stage_a(w + 2)
            stage_b(w)
            if w >= 1:
                stage_c(w - 1)
            if w >= 2:
                stage_d(w - 2)
        stage_c(NWL - 1)
        stage_d(NWL - 2)
        stage_d(NWL - 1)

    nc.compile()
    return nc


_NC_CACHE = {}


def _get_nc(cfg):
    k = cfg.key()
    if k not in _NC_CACHE:
        _NC_CACHE[k] = _build_nc(cfg)
    return _NC_CACHE[k]


def _emulate_core(cfg, im, Wx, vTm):
    """Numpy mirror of the device program (for offline validation)."""
    import ml_dtypes

    bf16 = ml_dtypes.bfloat16
    NWL, CS = cfg.NWL, cfg.CS
    H = H_HEADS
    Wxf = Wx.astype(np.float32)
    vTf = vTm.astype(np.float32)
    selfr = (im["xsT"].astype(np.float32).T @ Wxf)      # [PCR, 80]
    out = np.zeros((cfg.PCR, HC), np.float32)
    cb = 0
    for w in range(NWL):
        C = CS[w]
        xe = im["xeT"][:, cb * P:(cb + C) * P].astype(np.float32)
        ea = im["eaT"][:, cb * P:(cb + C) * P].astype(np.float32)
        ps = (xe.T @ Wxf[:, :XU]).reshape(C, P, XU)
        aed = (ea.T @ vTf).reshape(C, P, 2 * H)
        ps[:, :, HC:XU] += aed                          # psum accumulate
        selfw = selfr[w * P:(w + 1) * P]
        al = ps[:, :, HC:HC + H] + selfw[None, :, HC + 2 * H:HC + 3 * H]
        aes = ps[:, :, HC + H:XU].sum(axis=0)           # pure a_edge sums
        ex = np.exp(np.maximum(NEG_SLOPE * al, al)).astype(bf16)
        den = ex.astype(np.float32).sum(axis=0)
        xh_b = ps[:, :, :HC].astype(bf16).astype(np.float32)
        mw = (xh_b.reshape(C, P, H, C_OUT)
              * ex.astype(np.float32)[:, :, :, None]).astype(bf16)
        acc = mw.astype(np.float32).sum(axis=0).reshape(P, HC)
        lae = aes * im["invc"][:, w][:, None]
        asf = selfw[:, HC + 3 * H:W80] + lae
        es = np.exp(np.maximum(NEG_SLOPE * asf, asf))
        dent = den + es
        ot = (selfw[:, :HC].reshape(P, H, C_OUT) * es[:, :, None]
              + acc.reshape(P, H, C_OUT)) / dent[:, :, None]
        out[w * P:(w + 1) * P] = ot.reshape(P, HC)
        cb += C
    return out


def _emulate(cfg, in_maps, Wx, vTm):
    outs = [_emulate_core(cfg, im, Wx, vTm) for im in in_maps]
    return np.concatenate(outs, axis=0)


def kernel(**inputs):
    from concourse import mybir

    bf16 = mybir.dt.np(mybir.dt.bfloat16)
    x = np.asarray(inputs["x"], dtype=np.float32)
    ei = np.asarray(inputs["edge_index"])
    ea = np.asarray(inputs["edge_attr"], dtype=np.float32)
    W = np.asarray(inputs["W"], dtype=np.float32)
    W_edge = np.asarray(inputs["W_edge"], dtype=np.float32)
    att_src = np.asarray(inputs["att_src"], dtype=np.float32)
    att_dst = np.asarray(inputs["att_dst"], dtype=np.float32)
    att_edge = np.asarray(inputs["att_edge"], dtype=np.float32)
    bias = np.asarray(inputs["bias"], dtype=np.float32)

    src = ei[0].astype(np.int64)
    dst = ei[1].astype(np.int64)
    Wx, vTm = _fold_weights(W, W_edge, att_src, att_dst, att_edge)

    cfg, in_maps, meta = _prep(x, src, dst, ea)
    Wx_bf = np.ascontiguousarray(Wx.astype(bf16))
    vTm_bf = np.ascontiguousarray(vTm.astype(bf16))
    for im in in_maps:
        im["Wx"] = Wx_bf
        im["vTm"] = vTm_bf

    nc = _get_nc(cfg)

    from concourse.bass_utils import run_bass_kernel_spmd
    res = run_bass_kernel_spmd(nc, in_maps, core_ids=list(range(NCORES)),
                               trace=TRACE)
    if TRACE:
        global LAST_RESULT
        LAST_RESULT = res

    out_ws = np.concatenate([res.results[c]["out"] for c in range(NCORES)],
                            axis=0)  # [NCORES*PCR, HC] in window space
    out = out_ws[meta["winpos"]]
    return (out + bias[None, :]).astype(np.float32)


# revision 51
# speedup vs baseline: 1.1683x; 1.1683x over previous
"""GAT message-passing kernel for Trainium2, 8 NeuronCores, dst-aligned.

Strategy (self-contained; sized for N=50000, E=800000, D=128, H=4, C=16,
ED=64 but parameterized):
 - Nodes are sorted by in-degree and packed 128-consecutive into windows,
   so a window's max degree ~= its mean degree.  Window w's edges live in
   a [128 partitions x C_w columns] slot grid: partition = destination
   node's slot, column = edge ordinal.  Windows are dealt round-robin to
   the 8 cores with an equalized column schedule CS (max over the 8 cores
   at each rank) so every core runs the identical SPMD program.
 - The host ships, per edge slot, the source node's raw features x[src]
   (bf16, transposed) and edge_attr plus a mask row (-256 for padded
   slots).  No device-side gather, no index tensors: the layout IS the
   graph.
 - Per 128-edge block two bf16 matmuls write one PSUM tile: [xh | a_src]
   = x_src @ [W.T|u_src.T|0] and, accumulated onto cols 64:72,
   [v.ea+mask | v.ea] (mask row kills padded slots).  a_dst is a
   per-partition broadcast from the core's own node table.  alpha =
   lrelu(a_src+a_dst+a_edge) -> exp (ScalarE); softmax denominator and
   sum-of-a_edge via single 3D tensor_reduce ops; messages = one
   broadcast multiply + a halving-tree reduction, all packed bf16 SBUF.
 - The emission order is software-pipelined (stage B(w) on PE/ACT, stage
   C(w-1) alpha/exp, stage D(w-2) messages/close) so no in-order engine
   queue head-blocks on a cross-engine dependency.
 - Self-loops (PyG GATConv default: loop edge_attr = per-dst mean of
   incoming edge_attr) close each window via the pure sum-of-a_edge.
"""

import math

import numpy as np

NCORES = 8
D_IN = 128
H_HEADS = 4
C_OUT = 16
HC = H_HEADS * C_OUT  # 64
ED_DIM = 64
EAR = ED_DIM + 1      # edge-attr rows + mask row
NEG_SLOPE = 0.2
MASKV = -256.0        # padded slots: exp(lrelu(-256+...)) == 0
XU = 72               # psum row: [xh(64) | asrc+aedge+mask(4) | aedge(4)]
W80 = 80              # Wx cols: [W | u_src | 0 | u_dst | u_src+u_dst]

P = 128

TRACE = False       # set by test harness to capture an NTFF profile
LAST_RESULT = None  # BassKernelResults of the last traced run


class _Cfg:
    def __init__(self, nwl, cs):
        self.NWL = nwl            # windows per core
        self.CS = tuple(cs)       # 128-edge blocks per window (shared SPMD)
        self.TOTB = sum(cs)       # total blocks per core
        self.CMAX = max(cs)
        self.PCR = nwl * P        # node slots per core

    def key(self):
        return (self.NWL, self.CS)


def _fold_weights(W, W_edge, att_src, att_dst, att_edge):
    H, C = att_src.shape
    D = W.shape[1]
    ED = W_edge.shape[1]
    u_src = np.einsum("hc,hcd->hd", att_src, W.reshape(H, C, D))
    u_dst = np.einsum("hc,hcd->hd", att_dst, W.reshape(H, C, D))
    v = np.einsum("hc,hcd->hd", att_edge, W_edge.reshape(H, C, ED))
    Wx = np.zeros((D, W80), np.float32)
    Wx[:, :HC] = W.T
    Wx[:, HC:HC + H] = u_src.T
    Wx[:, HC + 2 * H:HC + 3 * H] = u_dst.T
    Wx[:, HC + 3 * H:] = (u_src + u_dst).T
    # vTm: cols 0:4 = v.T with mask row MASKV; cols 4:8 = v.T with mask 0
    vTm = np.zeros((EAR, 2 * H), np.float32)
    vTm[:ED, :H] = v.T
    vTm[:ED, H:] = v.T
    vTm[ED, :H] = MASKV
    return Wx, vTm


def _prep(x, src, dst, edge_attr):
    """Degree-sorted dst-aligned slot layout; per-core input slabs."""
    from concourse import mybir

    bf16 = mybir.dt.np(mybir.dt.bfloat16)
    n = x.shape[0]
    nwl = math.ceil(n / (P * NCORES))
    nwin = NCORES * nwl

    deg = np.bincount(dst, minlength=n).astype(np.int64)
    order = np.argsort(-deg, kind="stable")
    node_win = np.empty(n, np.int32)
    node_slot = np.empty(n, np.int32)
    ranks = np.arange(n, dtype=np.int64)
    node_win[order] = (ranks // P).astype(np.int32)
    node_slot[order] = (ranks % P).astype(np.int32)
    # window w max degree = degree of its first (highest-degree) node
    wmax = deg[order[np.minimum(np.arange(nwin) * P, n - 1)]]
    cs = np.maximum(wmax[0::NCORES], 1).astype(np.int64)  # equalized (desc)
    cfg = _Cfg(nwl, [int(c) for c in cs])
    cb = np.zeros(nwl + 1, np.int64)
    np.cumsum(cs, out=cb[1:])
    totb = cfg.TOTB

    core_of_win = np.arange(nwin) % NCORES
    lw_of_win = np.arange(nwin) // NCORES

    # edge -> (core, column in core slab, partition)
    ew = node_win[dst]
    ep = node_slot[dst]
    eorder = np.argsort(dst, kind="stable")
    ds = dst[eorder]
    first = np.zeros(len(ds), bool)
    first[0] = True
    first[1:] = ds[1:] != ds[:-1]
    gidx = np.flatnonzero(first)
    ec = np.arange(len(ds), dtype=np.int64)
    ec -= np.repeat(ec[gidx], np.diff(np.append(gidx, len(ds))))
    ecol = np.empty(len(ds), np.int64)
    ecol[eorder] = ec                                  # ordinal within dst
    ecore = core_of_win[ew]
    eslab = (cb[lw_of_win[ew]] + ecol) * P + ep        # column in core slab

    x_bf = np.ascontiguousarray(x.astype(bf16))
    ea_bf = np.ascontiguousarray(edge_attr.astype(bf16))

    in_maps = []
    for c in range(NCORES):
        em = ecore == c
        cols = eslab[em]
        xe = np.zeros((totb * P, D_IN), bf16)
        xe[cols] = x_bf[src[em]]
        xeT = np.ascontiguousarray(xe.T)
        eat = np.zeros((EAR, totb * P), bf16)
        eat[ED_DIM, :] = 1.0
        ea_blk = np.zeros((totb * P, ED_DIM), bf16)
        ea_blk[cols] = ea_bf[em]
        eat[:ED_DIM] = ea_blk.T
        eat[ED_DIM, cols] = 0.0

        wins = np.flatnonzero(core_of_win == c)        # in lw order
        xs = np.zeros((cfg.PCR, D_IN), bf16)
        invc = np.ones((P, nwl), np.float32)
        for lw, w in enumerate(wins):
            base = w * P
            cnt = min(P, n - base) if base < n else 0
            nd = order[base:base + cnt]
            xs[lw * P:lw * P + cnt] = x_bf[nd]
            invc[:cnt, lw] = 1.0 / np.maximum(deg[nd], 1.0)
        xsT = np.ascontiguousarray(xs.T)
        in_maps.append(dict(xeT=xeT, eaT=eat, xsT=xsT,
                            invc=np.ascontiguousarray(invc)))

    winpos = (core_of_win[node_win].astype(np.int64) * cfg.PCR
              + lw_of_win[node_win].astype(np.int64) * P + node_slot)
    meta = dict(winpos=winpos)
    return cfg, in_maps, meta


def _build_nc(cfg):
    import concourse.tile as tile
    from concourse import bacc, mybir
    from contextlib import ExitStack

    f32 = mybir.dt.float32
    bf16 = mybir.dt.bfloat16
    NWL, CS, TOTB, CMAX, PCR = cfg.NWL, cfg.CS, cfg.TOTB, cfg.CMAX, cfg.PCR
    UH = H_HEADS
    G = 7  # blocks per PSUM group: 7*72*4B = 2016B, one bank

    nc = bacc.Bacc("TRN2", target_bir_lowering=False, debug=False,
                   num_devices=NCORES)
    xeT = nc.dram_tensor("xeT", [P, TOTB * P], bf16, kind="ExternalInput").ap()
    eaT = nc.dram_tensor("eaT", [EAR, TOTB * P], bf16,
                         kind="ExternalInput").ap()
    xsT = nc.dram_tensor("xsT", [P, PCR], bf16, kind="ExternalInput").ap()
    Wx = nc.dram_tensor("Wx", [P, W80], bf16, kind="ExternalInput").ap()
    vTm = nc.dram_tensor("vTm", [EAR, 2 * UH], bf16,
                         kind="ExternalInput").ap()
    invc = nc.dram_tensor("invc", [P, NWL], f32, kind="ExternalInput").ap()
    out = nc.dram_tensor("out", [PCR, HC], f32, kind="ExternalOutput").ap()

    AF = mybir.ActivationFunctionType
    ALU = mybir.AluOpType
    AX = mybir.AxisListType

    with tile.TileContext(nc) as tc, ExitStack() as ctx:
        cpool = ctx.enter_context(tc.tile_pool(name="const", bufs=1))
        selfpool = ctx.enter_context(tc.tile_pool(name="selfr", bufs=1))
        xspool = ctx.enter_context(tc.tile_pool(name="xs", bufs=1))
        xepool = ctx.enter_context(tc.tile_pool(name="xe", bufs=3))
        eapool = ctx.enter_context(tc.tile_pool(name="ea", bufs=3))
        xhpool = ctx.enter_context(tc.tile_pool(name="xh", bufs=3))
        alfpool = ctx.enter_context(tc.tile_pool(name="alf", bufs=3))
        wpool = ctx.enter_context(tc.tile_pool(name="win", bufs=3))
        mpool = ctx.enter_context(tc.tile_pool(name="m", bufs=2))
        opool = ctx.enter_context(tc.tile_pool(name="o", bufs=3))
        psself = ctx.enter_context(
            tc.tile_pool(name="ps_s", bufs=2, space="PSUM"))
        psx = ctx.enter_context(tc.tile_pool(name="ps_x", bufs=4,
                                             space="PSUM"))
        psacc = ctx.enter_context(tc.tile_pool(name="ps_acc", bufs=2,
                                               space="PSUM"))

        Wx_sb = cpool.tile([P, W80], bf16)
        nc.sync.dma_start(Wx_sb[:], Wx[:])
        from concourse.masks import make_identity
        ident_sb = cpool.tile([P, P], bf16)
        make_identity(nc, ident_sb[:])
        vTm_sb = cpool.tile([EAR, 2 * UH], bf16)
        nc.sync.dma_start(vTm_sb[:], vTm[:])
        invc_sb = cpool.tile([P, NWL], f32)
        nc.sync.dma_start(invc_sb[:], invc[:])
        selfr = selfpool.tile([P, NWL * W80], f32)
        xs = xspool.tile([P, PCR], bf16)

        # ---- software-pipelined main loop ----
        cbs = [0]
        for c in CS:
            cbs.append(cbs[-1] + c)
        state = {}

        def stage_a(w):  # input slab DMA
            C = CS[w]
            cb = cbs[w]
            xe = xepool.tile([P, CMAX * P], bf16, tag="xe")
            nc.sync.dma_start(xe[:, :C * P], xeT[:, cb * P:(cb + C) * P])
            ea = eapool.tile([EAR, CMAX * P], bf16, tag="ea")
            nc.gpsimd.dma_start(ea[:, :C * P], eaT[:, cb * P:(cb + C) * P])
            state[w] = dict(xe=xe, ea=ea)

        def stage_b(w):  # matmuls + PSUM->SBUF copies
            C = CS[w]
            st = state[w]
            xe, ea = st["xe"], st["ea"]
            xhs = xhpool.tile([P, CMAX * HC], bf16, tag="xhs")
            alf = alfpool.tile([P, CMAX * 2 * UH], f32, tag="alf")
            for g in range(math.ceil(C / G)):
                c0 = g * G
                ng = min(G, C - c0)
                ps = psx.tile([P, G * XU], f32)
                for i in range(ng):
                    c = c0 + i
                    nc.tensor.matmul(
                        out=ps[:, i * XU:i * XU + XU],
                        lhsT=xe[:, c * P:(c + 1) * P],
                        rhs=Wx_sb[:, 0:XU], start=True, stop=False,
                        skip_group_check=True)
                    nc.tensor.matmul(
                        out=ps[:, i * XU + HC:i * XU + XU],
                        lhsT=ea[:, c * P:(c + 1) * P],
                        rhs=vTm_sb[:], start=False, stop=True,
                        skip_group_check=True)
                psv = ps[:, :ng * XU].rearrange("p (c u) -> p c u", u=XU)
                nc.scalar.activation(
                    xhs[:, c0 * HC:(c0 + ng) * HC], psv[:, :, 0:HC],
                    AF.Copy)
                nc.scalar.activation(
                    alf[:, c0 * 2 * UH:(c0 + ng) * 2 * UH],
                    psv[:, :, HC:XU], AF.Copy)
            pss = psself.tile([P, W80], f32)
            nc.tensor.matmul(out=pss[:], lhsT=xs[:, w * P:(w + 1) * P],
                             rhs=Wx_sb[:], start=True, stop=True)
            nc.scalar.activation(selfr[:, w * W80:(w + 1) * W80], pss[:],
                                 AF.Copy)
            st["xhs"] = xhs
            st["alf"] = alf

        def stage_c(w):  # alpha: +adst, lrelu, exp, den, aes
            C = CS[w]
            st = state[w]
            alf = st["alf"]
            selfw = selfr[:, w * W80:(w + 1) * W80]
            al8 = alf[:, :2 * UH * C].rearrange("p (c h) -> p c h",
                                                h=2 * UH)
            nc.gpsimd.tensor_tensor(
                out=al8[:, :, 0:UH], in0=al8[:, :, 0:UH],
                in1=selfw[:, HC + 2 * UH:HC + 3 * UH].unsqueeze(1)
                .broadcast_to([P, C, UH]), op=ALU.add)
            lrt = wpool.tile([P, CMAX * UH], f32, tag="lrt")
            lrtv = lrt[:, :UH * C].rearrange("p (c h) -> p c h", h=UH)
            nc.vector.scalar_tensor_tensor(
                out=lrtv, in0=al8[:, :, 0:UH], scalar=NEG_SLOPE,
                in1=al8[:, :, 0:UH], op0=ALU.mult, op1=ALU.max)
            expal = wpool.tile([P, CMAX * UH], bf16, tag="expal")
            nc.scalar.activation(expal[:, :UH * C], lrt[:, :UH * C], AF.Exp)
            den = wpool.tile([P, UH], f32, tag="den")
            nc.vector.tensor_reduce(
                den[:], expal[:, :UH * C].rearrange("p (c h) -> p h c",
                                                    h=UH),
                axis=AX.X, op=ALU.add)
            aes = wpool.tile([P, UH], f32, tag="aes")
            nc.vector.tensor_reduce(
                aes[:], al8[:, :, UH:2 * UH].rearrange("p c h -> p h c"),
                axis=AX.X, op=ALU.add)
            st["expal"] = expal
            st["den"] = den
            st["aes"] = aes

        def stage_d(w):  # messages + close
            C = CS[w]
            st = state.pop(w)
            xhs, expal = st["xhs"], st["expal"]
            den, aes = st["den"], st["aes"]
            selfw = selfr[:, w * W80:(w + 1) * W80]
            mw = mpool.tile([P, CMAX * HC], bf16, tag="mw")
            nc.vector.tensor_tensor(
                out=mw[:, :C * HC].rearrange("p (c h u) -> p c h u",
                                             h=UH, u=C_OUT),
                in0=xhs[:, :C * HC].rearrange("p (c h u) -> p c h u",
                                              h=UH, u=C_OUT),
                in1=expal[:, :UH * C].rearrange("p (c h) -> p c h", h=UH)
                .unsqueeze(3).broadcast_to([P, C, UH, C_OUT]),
                op=ALU.mult)
            JA = min(4, C)
            nga = math.ceil(C / 4)
            psa = psacc.tile([P, 4 * HC], f32)
            for g in range(nga):
                c0 = 4 * g
                nq = min(4, C - c0)
                nc.tensor.matmul(
                    out=psa[:, :nq * HC], lhsT=ident_sb[:],
                    rhs=mw[:, c0 * HC:(c0 + nq) * HC],
                    start=(g == 0), stop=(g == nga - 1),
                    skip_group_check=True)
            acc = opool.tile([P, HC], f32, tag="acc")
            if JA == 1:
                nc.vector.tensor_copy(acc[:], psa[:, :HC])
            else:
                nc.vector.tensor_reduce(
                    acc[:], psa[:, :JA * HC].rearrange("p (j u) -> p u j",
                                                       u=HC),
                    axis=AX.X, op=ALU.add)

            lae = wpool.tile([P, UH], f32, tag="lae")
            nc.vector.tensor_scalar(
                out=lae[:], in0=aes[:], scalar1=invc_sb[:, w:w + 1],
                scalar2=None, op0=ALU.mult)
            asf = wpool.tile([P, UH], f32, tag="asf")
            nc.gpsimd.tensor_tensor(out=asf[:],
                                    in0=selfw[:, HC + 3 * UH:W80],
                                    in1=lae[:], op=ALU.add)
            es = wpool.tile([P, UH], f32, tag="es")
            nc.vector.scalar_tensor_tensor(
                out=es[:], in0=asf[:], scalar=NEG_SLOPE, in1=asf[:],
                op0=ALU.mult, op1=ALU.max)
            nc.scalar.activation(es[:], es[:], AF.Exp)
            dent = wpool.tile([P, UH], f32, tag="dent")
            nc.gpsimd.tensor_tensor(out=dent[:], in0=es[:], in1=den[:],
                                    op=ALU.add)
            rec = wpool.tile([P, UH], f32, tag="rec")
            nc.vector.reciprocal(rec[:], dent[:])
            ot = opool.tile([P, HC], f32, tag="ot")
            nc.vector.tensor_tensor(
                out=ot[:].rearrange("p (h u) -> p h u", u=C_OUT),
                in0=selfw[:, 0:HC].rearrange("p (h u) -> p h u", u=C_OUT),
                in1=es[:].unsqueeze(2).broadcast_to([P, UH, C_OUT]),
                op=ALU.mult)
            nc.gpsimd.tensor_tensor(out=ot[:], in0=ot[:], in1=acc[:],
                                    op=ALU.add)
            nc.gpsimd.tensor_tensor(
                out=ot[:].rearrange("p (h u) -> p h u", u=C_OUT),
                in0=ot[:].rearrange("p (h u) -> p h u", u=C_OUT),
                in1=rec[:].unsqueeze(2).broadcast_to([P, UH, C_OUT]),
                op=ALU.mult)
            nc.sync.dma_start(out[w * P:(w + 1) * P, :], ot[:])

        stage_a(0)
        if NWL > 1:
            stage_a(1)
        nc.sync.dma_start(xs[:], xsT[:])
        for w in range(NWL):
            if w + 2 < NWL:
                stage_a(w + 2)
            stage_b(w)
            if w >= 1:
                stage_c(w - 1)
            if w >= 2:
                stage_d(w - 2)
        stage_c(NWL - 1)
        stage_d(NWL - 2)
        stage_d(NWL - 1)

    nc.compile()
    return nc


_NC_CACHE = {}


def _get_nc(cfg):
    k = cfg.key()
    if k not in _NC_CACHE:
        _NC_CACHE[k] = _build_nc(cfg)
    return _NC_CACHE[k]


def _emulate_core(cfg, im, Wx, vTm):
    """Numpy mirror of the device program (for offline validation)."""
    import ml_dtypes

    bf16 = ml_dtypes.bfloat16
    NWL, CS = cfg.NWL, cfg.CS
    H = H_HEADS
    Wxf = Wx.astype(np.float32)
    vTf = vTm.astype(np.float32)
    selfr = (im["xsT"].astype(np.float32).T @ Wxf)      # [PCR, 80]
    out = np.zeros((cfg.PCR, HC), np.float32)
    cb = 0
    for w in range(NWL):
        C = CS[w]
        xe = im["xeT"][:, cb * P:(cb + C) * P].astype(np.float32)
        ea = im["eaT"][:, cb * P:(cb + C) * P].astype(np.float32)
        ps = (xe.T @ Wxf[:, :XU]).reshape(C, P, XU)
        aed = (ea.T @ vTf).reshape(C, P, 2 * H)
        ps[:, :, HC:XU] += aed                          # psum accumulate
        selfw = selfr[w * P:(w + 1) * P]
        al = ps[:, :, HC:HC + H] + selfw[None, :, HC + 2 * H:HC + 3 * H]
        aes = ps[:, :, HC + H:XU].sum(axis=0)           # pure a_edge sums
        ex = np.exp(np.maximum(NEG_SLOPE * al, al)).astype(bf16)
        den = ex.astype(np.float32).sum(axis=0)
        xh_b = ps[:, :, :HC].astype(bf16).astype(np.float32)
        mw = (xh_b.reshape(C, P, H, C_OUT)
              * ex.astype(np.float32)[:, :, :, None]).astype(bf16)
        acc = mw.astype(np.float32).sum(axis=0).reshape(P, HC)
        lae = aes * im["invc"][:, w][:, None]
        asf = selfw[:, HC + 3 * H:W80] + lae
        es = np.exp(np.maximum(NEG_SLOPE * asf, asf))
        dent = den + es
        ot = (selfw[:, :HC].reshape(P, H, C_OUT) * es[:, :, None]
              + acc.reshape(P, H, C_OUT)) / dent[:, :, None]
        out[w * P:(w + 1) * P] = ot.reshape(P, HC)
        cb += C
    return out


def _emulate(cfg, in_maps, Wx, vTm):
    outs = [_emulate_core(cfg, im, Wx, vTm) for im in in_maps]
    return np.concatenate(outs, axis=0)


def kernel(**inputs):
    from concourse import mybir

    bf16 = mybir.dt.np(mybir.dt.bfloat16)
    x = np.asarray(inputs["x"], dtype=np.float32)
    ei = np.asarray(inputs["edge_index"])
    ea = np.asarray(inputs["edge_attr"], dtype=np.float32)
    W = np.asarray(inputs["W"], dtype=np.float32)
    W_edge = np.asarray(inputs["W_edge"], dtype=np.float32)
    att_src = np.asarray(inputs["att_src"], dtype=np.float32)
    att_dst = np.asarray(inputs["att_dst"], dtype=np.float32)
    att_edge = np.asarray(inputs["att_edge"], dtype=np.float32)
    bias = np.asarray(inputs["bias"], dtype=np.float32)

    src = ei[0].astype(np.int64)
    dst = ei[1].astype(np.int64)
    Wx, vTm = _fold_weights(W, W_edge, att_src, att_dst, att_edge)

    cfg, in_maps, meta = _prep(x, src, dst, ea)
    Wx_bf = np.ascontiguousarray(Wx.astype(bf16))
    vTm_bf = np.ascontiguousarray(vTm.astype(bf16))
    for im in in_maps:
        im["Wx"] = Wx_bf
        im["vTm"] = vTm_bf

    nc = _get_nc(cfg)

    from concourse.bass_utils import run_bass_kernel_spmd
    res = run_bass_kernel_spmd(nc, in_maps, core_ids=list(range(NCORES)),
                               trace=TRACE)
    if TRACE:
        global LAST_RESULT
        LAST_RESULT = res

    out_ws = np.concatenate([res.results[c]["out"] for c in range(NCORES)],
                            axis=0)  # [NCORES*PCR, HC] in window space
    out = out_ws[meta["winpos"]]
    return (out + bias[None, :]).astype(np.float32)
